# revision 9
# baseline (speedup 1.0000x reference)
"""Trainium2 Bass kernel for nn_AttentionBridge (ALiBi cross-attention + out-proj + rmsnorm residual).

Sharding: 8 cores = 2 (batch) x 4 (head groups of 4 heads).
Core c: batch b = c//4, head group g = c%4 (global heads 4g..4g+3).

Math per core (all in the "transposed domain" so no activation transposes are
needed after the initial query^T/context^T):
  queryT/ctxT via PE transposes -> f32r projections
  qT[dout,t] = WqT.T @ queryT   (scale 1/8 folded in)
  kT[dout,s], v[s,dout] similarly
  scoresT[s,t] = kT_h.T @ qT_h  (K=64) + alibi bias (host-precomputed band tiles)
  expsT = exp(scoresT)          (banded: only |t-s| <= ~128..384 computed;
                                 ALiBi slopes 0.5..0.96 make the rest < e^-60)
  PV with lhsT = [v_h | ones]: psum rows 0:64 = out, rows 64:128 = softmax denom
  normalize -> outT_g slice for that t-block
  per t-block: local out-proj (Wo column slice) + chunked ReduceScatter(add)
  within the batch group (overlaps later t-blocks' attention), then
  residual + rmsnorm on this core's rows.
Host assembles [2,1024,1024] from the per-core row sets.
"""
import os
import sys

for _p in ("/opt/trn_rl_repo", "/root/.axon_site/_ro/trn_rl_repo"):
    if os.path.isdir(_p) and _p not in sys.path:
        sys.path.insert(0, _p)

import numpy as np

# ---- problem constants (hardcoded per contract) ----
B, T, S_FULL, D = 2, 1024, 2048, 1024
H, HD = 16, 64
NH = 4              # heads per core
DOUT = NH * HD      # 256 projected dims per core
SCALE = HD ** -0.5  # 0.125
EPS = 1e-6
N_CORES = 8
KSUB = D // 128     # 8 contraction subtiles
NST = 9             # context s-tiles kept (s < 1152; beyond that alibi bias < -400)
S_KEEP = NST * 128  # 1152
NTB = 4             # t-blocks of 256
TBW = 256
SLOPES = (1.0 / np.power(2.0, np.arange(1, H + 1, dtype=np.float64) / H)).astype(np.float32)

TRACE = False
LAST_EXEC_NS = None
_CACHE = {}


def _row_map(g: int) -> np.ndarray:
    """Global t row held at local row r (0..255) of the per-core result.

    Local rows are [mt 2][p 128]; chunk tb = 2*mt + p//64; within = p%64;
    global t = 256*tb + 64*g + within (g = rank in the 4-core RS group).
    """
    r = np.arange(2 * 128)
    mt, p = r // 128, r % 128
    tb = 2 * mt + p // 64
    return 256 * tb + 64 * g + (p % 64)


def _install_ntff_shim():
    """Optional: register the NTFF profile hook so trace=True works under axon."""
    import types
    try:
        from trn_agent_boot.trn_boot import _ntff_profile_via_ctypes
        hook = _ntff_profile_via_ctypes('/opt/axon/libaxon_pjrt.so')
    except Exception:
        hook = None
    mod = types.ModuleType('antenv.axon_hooks')
    mod.get_axon_ntff_profile_hook = lambda: hook
    mod.set_axon_ntff_profile_hook = lambda h: None
    sys.modules.setdefault('antenv.axon_hooks', mod)


def _build():
    import concourse.bass as bass
    import concourse.mybir as mybir
    import concourse.tile as tile
    from concourse import bacc
    from concourse.masks import make_identity

    f32 = mybir.dt.float32
    f32r = mybir.dt.float32r
    AF = mybir.ActivationFunctionType
    ALU = mybir.AluOpType

    nc = bacc.Bacc("TRN2", target_bir_lowering=False, debug=False, num_devices=N_CORES)

    q_in = nc.dram_tensor("q_in", [T, D], f32, kind="ExternalInput")
    c_in = nc.dram_tensor("c_in", [S_KEEP, D], f32, kind="ExternalInput")
    wq_in = nc.dram_tensor("wq_in", [DOUT, D], f32, kind="ExternalInput")
    wk_in = nc.dram_tensor("wk_in", [DOUT, D], f32, kind="ExternalInput")
    wv_in = nc.dram_tensor("wv_in", [DOUT, D], f32, kind="ExternalInput")
    wo_in = nc.dram_tensor("wo_in", [D, DOUT], f32, kind="ExternalInput")
    qres_in = nc.dram_tensor("qres_in", [TBW, D], f32, kind="ExternalInput")
    bias_in = nc.dram_tensor("bias_in", [NH, 4, 128, TBW], f32, kind="ExternalInput")
    rwb_in = nc.dram_tensor("rwb_in", [128, D], f32, kind="ExternalInput")
    y_out = nc.dram_tensor("y_out", [TBW, D], f32, kind="ExternalOutput")

    with tile.TileContext(nc) as tc:
        with tc.tile_pool(name="const", bufs=1) as cpool, \
             tc.tile_pool(name="srcp", bufs=4) as srcp, \
             tc.tile_pool(name="pers0", bufs=1) as pers0, \
             tc.tile_pool(name="dramp", bufs=1, space="DRAM") as dramp:

            ident = cpool.tile([128, 128], f32)
            make_identity(nc, ident)
            onesF = cpool.tile([128, 1], f32)
            nc.gpsimd.memset(onesF[:], 1.0)
            epsT = cpool.tile([128, 1], f32)
            nc.gpsimd.memset(epsT[:], EPS)

            woT = pers0.tile([128, 2, D], f32r)   # local slice: WoT[din_local 256, dout 1024]
            outT_sb = pers0.tile([128, 2, T], f32r)
            rwb = pers0.tile([128, D], f32)
            qres = pers0.tile([128, 2, D], f32)
            nc.sync.dma_start(rwb[:], rwb_in[:, :])
            nc.sync.dma_start(qres[:], qres_in.ap().rearrange("(m p) d -> p m d", p=128))

            rs_ins = [dramp.tile([TBW, D], f32, name=f"rs_in{t}") for t in range(NTB)]
            rs_outs = [dramp.tile([64, D], f32, name=f"rs_out{t}") for t in range(NTB)]

            def transpose_in(psT, dst, dram, nrows, ncols):
                # dst[p, k, r] = dram[r, 128k + p]; copies batched 4 transposes wide
                kgrp = 4 if ncols % 512 == 0 else 2
                for rt in range(nrows // 128):
                    st = srcp.tile([128, D], f32, tag="srcld", name="src_t")
                    nc.sync.dma_start(st[:, :ncols], dram[rt * 128:(rt + 1) * 128, :])
                    for k0 in range(0, ncols // 128, kgrp):
                        p = psT.tile([128, 512], f32, tag="tp", name="tp_ps")
                        for j in range(kgrp):
                            nc.tensor.transpose(p[:, j * 128:(j + 1) * 128],
                                                st[:, (k0 + j) * 128:(k0 + j + 1) * 128],
                                                ident[:])
                        nc.vector.tensor_copy(
                            dst[:, k0:k0 + kgrp, rt * 128:(rt + 1) * 128],
                            p[:, :kgrp * 128].rearrange("p (j r) -> p j r", r=128))

            with tc.tile_pool(name="persA", bufs=1) as persA:
                wqT = persA.tile([128, KSUB, DOUT], f32r)
                wkT = persA.tile([128, KSUB, DOUT], f32r)
                wvT = persA.tile([128, KSUB, DOUT], f32r)
                qT = persA.tile([128, 2, T], f32r)
                kT = persA.tile([128, 2, S_KEEP], f32r)
                vA = persA.tile([128, NST, NH, 128], f32r)   # [s | st | h | 64 v + 64 ones]
                bias_sb = persA.tile([128, NH, 4, TBW], f32)

                nc.sync.dma_start(bias_sb[:], bias_in.ap().rearrange("h d p j -> p h d j"))
                nc.vector.tensor_copy(vA[:, :, :, 64:128],
                                      onesF[:].to_broadcast((128, NST, NH, 64)))

                with tc.tile_pool(name="psT", bufs=2, space="PSUM") as psT:
                    transpose_in(psT, wqT, wq_in, DOUT, D)
                    transpose_in(psT, wkT, wk_in, DOUT, D)
                    transpose_in(psT, wvT, wv_in, DOUT, D)

                    with tc.tile_pool(name="bigT", bufs=1) as bigTp, \
                         tc.tile_pool(name="psP", bufs=2, space="PSUM") as psP:
                        bigQ = bigTp.tile([128, KSUB, S_KEEP], f32r, tag="big", name="bigQ")
                        transpose_in(psT, bigQ, q_in, T, D)  # uses [:, :, :1024]
                        # q projection (scale folded in)
                        for sub in range(2):
                            for nb in range(2):
                                pp = psP.tile([128, 512], f32, tag="proj", name="pp_q")
                                for k in range(KSUB):
                                    nc.tensor.matmul(pp[:], wqT[:, k, sub * 128:(sub + 1) * 128],
                                                     bigQ[:, k, nb * 512:(nb + 1) * 512],
                                                     start=(k == 0), stop=(k == KSUB - 1))
                                nc.scalar.mul(qT[:, sub, nb * 512:(nb + 1) * 512], pp[:], SCALE)

                        bigC = bigTp.tile([128, KSUB, S_KEEP], f32r, tag="big", name="bigC")
                        transpose_in(psT, bigC, c_in, S_KEEP, D)
                        # k projection
                        for sub in range(2):
                            for nb, (n0, nw) in enumerate(((0, 512), (512, 512), (1024, 128))):
                                pp = psP.tile([128, 512], f32, tag="proj", name="pp_k")
                                for k in range(KSUB):
                                    nc.tensor.matmul(pp[:, :nw], wkT[:, k, sub * 128:(sub + 1) * 128],
                                                     bigC[:, k, n0:n0 + nw],
                                                     start=(k == 0), stop=(k == KSUB - 1))
                                nc.any.tensor_copy(kT[:, sub, n0:n0 + nw], pp[:, :nw])
                        # v projection (natural layout, into vA's v-halves)
                        for st in range(NST):
                            pp = psP.tile([128, 512], f32, tag="proj", name="pp_v")
                            for k in range(KSUB):
                                nc.tensor.matmul(pp[:, :DOUT], bigC[:, k, st * 128:(st + 1) * 128],
                                                 wvT[:, k, :], start=(k == 0), stop=(k == KSUB - 1))
                            nc.any.tensor_copy(vA[:, st, :, 0:64],
                                               pp[:, :DOUT].rearrange("p (h e) -> p h e", e=64))

                    # Wo slice transposes (before attention; scheduler overlaps)
                    transpose_in(psT, woT, wo_in, D, DOUT)

                # ---- attention (tb-outer) + per-tb out-proj + chunked ReduceScatter ----
                with tc.tile_pool(name="expp", bufs=3) as expp, \
                     tc.tile_pool(name="tmpB", bufs=2) as tmpB, \
                     tc.tile_pool(name="stg", bufs=3) as stg, \
                     tc.tile_pool(name="psB", bufs=2, space="PSUM") as psB, \
                     tc.tile_pool(name="psO", bufs=2, space="PSUM") as psO:
                    for tb in range(NTB):
                        sts = [st for st in range(2 * tb - 1, 2 * tb + 3) if st >= 0]
                        nst = len(sts)
                        d0 = sts[0] - (2 * tb - 1)
                        for h in range(NH):
                            r0 = 64 * (h % 2)
                            sub = h // 2
                            ps_sc = psB.tile([128, 4, TBW], f32, tag="sc", name="ps_sc")
                            for i, st in enumerate(sts):
                                nc.tensor.matmul(ps_sc[:, i, :],
                                                 kT[r0:r0 + 64, sub, st * 128:(st + 1) * 128],
                                                 qT[r0:r0 + 64, sub, tb * TBW:(tb + 1) * TBW],
                                                 start=True, stop=True)
                            nc.vector.tensor_tensor(ps_sc[:, :nst], ps_sc[:, :nst],
                                                    bias_sb[:, h, d0:d0 + nst, :], ALU.add)
                            exps = expp.tile([128, 4, TBW], f32r, tag="exps", name="exps")
                            nc.scalar.activation(exps[:, :nst], ps_sc[:, :nst], AF.Exp)
                            po = psB.tile([128, TBW], f32, tag="po", name="po")
                            for i, st in enumerate(sts):
                                nc.tensor.matmul(po[:], vA[:, st, h, :], exps[:, i, :],
                                                 start=(i == 0), stop=(i == nst - 1))
                            rB = tmpB.tile([64, TBW], f32, tag="rB", name="rB")
                            nc.vector.reciprocal(rB[:], po[64:128, :])
                            nc.vector.tensor_tensor(outT_sb[r0:r0 + 64, sub, tb * TBW:(tb + 1) * TBW],
                                                    po[0:64, :], rB[:], ALU.mult)
                        # local out-proj for this t-block, then ReduceScatter chunk
                        for mt in range(2):
                            tt = 2 * tb + mt
                            for nb in range(2):
                                pp = psO.tile([128, 512], f32, tag="oproj", name="pp_o")
                                for k in range(2):
                                    nc.tensor.matmul(pp[:], outT_sb[:, k, tt * 128:(tt + 1) * 128],
                                                     woT[:, k, nb * 512:(nb + 1) * 512],
                                                     start=(k == 0), stop=(k == 1))
                                so = stg.tile([128, 512], f32, tag="postg", name="so")
                                nc.vector.tensor_copy(so[:], pp[:])
                                nc.sync.dma_start(
                                    rs_ins[tb][mt * 128:(mt + 1) * 128, nb * 512:(nb + 1) * 512],
                                    so[:])
                        nc.gpsimd.collective_compute(
                            "ReduceScatter", mybir.AluOpType.add,
                            ins=[rs_ins[tb].opt()], outs=[rs_outs[tb].opt()],
                            replica_groups=[[0, 1, 2, 3], [4, 5, 6, 7]])

            # ---- gather RS chunks, residual + rmsnorm ----
            with tc.tile_pool(name="persC", bufs=1) as persC:
                rq = persC.tile([128, 2, D], f32)
                for tb in range(NTB):
                    nc.sync.dma_start(rq[64 * (tb % 2):64 * (tb % 2) + 64, tb // 2, :],
                                      rs_outs[tb][:, :])
                for mt in range(2):
                    r = persC.tile([128, D], f32, tag="resid", name="resid")
                    nc.vector.tensor_tensor(r[:], rq[:, mt, :], qres[:, mt, :], ALU.add)
                    sq = persC.tile([128, D], f32, tag="sq", name="sq")
                    ms = persC.tile([128, 1], f32, tag="ms", name="ms")
                    nc.scalar.activation(sq[:], r[:], AF.Square, accum_out=ms[:])
                    rstd = persC.tile([128, 1], f32, tag="rstd", name="rstd")
                    nc.scalar.activation(rstd[:], ms[:], AF.Sqrt, scale=1.0 / D, bias=epsT[:])
                    rinv = persC.tile([128, 1], f32, tag="rinv", name="rinv")
                    nc.vector.reciprocal(rinv[:], rstd[:])
                    y1 = persC.tile([128, D], f32, tag="y1", name="y1")
                    nc.vector.tensor_scalar_mul(y1[:], r[:], rinv[:])
                    y2 = persC.tile([128, D], f32, tag="y2", name="y2")
                    nc.vector.tensor_tensor(y2[:], y1[:], rwb[:], ALU.mult)
                    nc.sync.dma_start(y_out[mt * 128:(mt + 1) * 128, :], y2[:])

    nc.compile()
    return nc


def _bias_tiles(g: int) -> np.ndarray:
    """Per-core host-precomputed alibi band tiles [NH, 4 deltas, 128, TBW]."""
    j = np.arange(TBW, dtype=np.float64)[None, :]
    p = np.arange(128, dtype=np.float64)[:, None]
    out = np.empty((NH, 4, 128, TBW), dtype=np.float32)
    for hl in range(NH):
        slope = float(SLOPES[4 * g + hl])
        for d in range(4):
            delta = 128 * (d - 1)
            out[hl, d] = (-slope * np.abs(j - p - delta)).astype(np.float32)
    return out


def kernel(query, context, Wq, Wk, Wv, Wo, rms_weight):
    global LAST_EXEC_NS
    query = np.ascontiguousarray(np.asarray(query, dtype=np.float32))
    context = np.ascontiguousarray(np.asarray(context, dtype=np.float32))
    Wq = np.ascontiguousarray(np.asarray(Wq, dtype=np.float32))
    Wk = np.ascontiguousarray(np.asarray(Wk, dtype=np.float32))
    Wv = np.ascontiguousarray(np.asarray(Wv, dtype=np.float32))
    Wo = np.ascontiguousarray(np.asarray(Wo, dtype=np.float32))
    rms_weight = np.asarray(rms_weight, dtype=np.float32)

    if TRACE:
        _install_ntff_shim()
    if "nc" not in _CACHE:
        _CACHE["nc"] = _build()
    nc = _CACHE["nc"]

    from concourse.bass_utils import run_bass_kernel_spmd

    rwb = np.ascontiguousarray(np.broadcast_to(rms_weight, (128, D))).astype(np.float32)
    in_maps = []
    for c in range(N_CORES):
        b, g = divmod(c, 4)
        rows = _row_map(g)
        in_maps.append({
            "q_in": query[b],
            "c_in": context[b, :S_KEEP],
            "wq_in": Wq[DOUT * g:DOUT * (g + 1)],
            "wk_in": Wk[DOUT * g:DOUT * (g + 1)],
            "wv_in": Wv[DOUT * g:DOUT * (g + 1)],
            "wo_in": np.ascontiguousarray(Wo[:, DOUT * g:DOUT * (g + 1)]),
            "qres_in": np.ascontiguousarray(query[b, rows]),
            "bias_in": _bias_tiles(g),
            "rwb_in": rwb,
        })

    res = run_bass_kernel_spmd(nc, in_maps, core_ids=list(range(N_CORES)), trace=TRACE)
    LAST_EXEC_NS = res.exec_time_ns
    _CACHE["last_result"] = res

    out = np.empty((B, T, D), dtype=np.float32)
    for c in range(N_CORES):
        b, g = divmod(c, 4)
        out[b, _row_map(g), :] = res.results[c]["y_out"]
    return out


# revision 16
# speedup vs baseline: 9363.0091x; 9363.0091x over previous
"""Trainium2 Bass kernel for nn_AttentionBridge (ALiBi cross-attention + out-proj + rmsnorm residual).

Sharding: 8 cores = 2 (batch) x 4 (head groups of 4 heads).
Core c: batch b = c//4, head group g = c%4 (global heads 4g..4g+3).

Math per core (all in the "transposed domain" so no activation transposes are
needed after the initial query^T/context^T):
  queryT/ctxT via PE transposes -> f32r projections
  qT[dout,t] = WqT.T @ queryT   (scale 1/8 folded in)
  kT[dout,s], v[s,dout] similarly
  scoresT[s,t] = kT_h.T @ qT_h  (K=64) + alibi bias (host-precomputed band tiles)
  expsT = exp(scoresT)          (banded: only |t-s| <= ~128..384 computed;
                                 ALiBi slopes 0.5..0.96 make the rest < e^-60)
  PV with lhsT = [v_h | ones]: psum rows 0:64 = out, rows 64:128 = softmax denom
  normalize -> outT_g slice for that t-block
  per t-block: local out-proj (Wo column slice) + chunked ReduceScatter(add)
  within the batch group (overlaps later t-blocks' attention), then
  residual + rmsnorm on this core's rows.
Host assembles [2,1024,1024] from the per-core row sets.
"""
import os
import sys

for _p in ("/opt/trn_rl_repo", "/root/.axon_site/_ro/trn_rl_repo"):
    if os.path.isdir(_p) and _p not in sys.path:
        sys.path.insert(0, _p)

import numpy as np

# ---- problem constants (hardcoded per contract) ----
B, T, S_FULL, D = 2, 1024, 2048, 1024
H, HD = 16, 64
NH = 4              # heads per core
DOUT = NH * HD      # 256 projected dims per core
SCALE = HD ** -0.5  # 0.125
EPS = 1e-6
N_CORES = 8
KSUB = D // 128     # 8 contraction subtiles
NST = 9             # context s-tiles kept (s < 1152; beyond that alibi bias < -400)
S_KEEP = NST * 128  # 1152
NTB = 4             # t-blocks of 256
TBW = 256
SLOPES = (1.0 / np.power(2.0, np.arange(1, H + 1, dtype=np.float64) / H)).astype(np.float32)

TRACE = False
LAST_EXEC_NS = None
_CACHE = {}


def _row_map(g: int) -> np.ndarray:
    """Global t row held at local row r (0..255) of the per-core result.

    Local rows are [mt 2][p 128]; chunk tb = 2*mt + p//64; within = p%64;
    global t = 256*tb + 64*g + within (g = rank in the 4-core RS group).
    """
    r = np.arange(2 * 128)
    mt, p = r // 128, r % 128
    tb = 2 * mt + p // 64
    return 256 * tb + 64 * g + (p % 64)


def _install_ntff_shim():
    """Optional: register the NTFF profile hook so trace=True works under axon."""
    import types
    try:
        from trn_agent_boot.trn_boot import _ntff_profile_via_ctypes
        hook = _ntff_profile_via_ctypes('/opt/axon/libaxon_pjrt.so')
    except Exception:
        hook = None
    mod = types.ModuleType('antenv.axon_hooks')
    mod.get_axon_ntff_profile_hook = lambda: hook
    mod.set_axon_ntff_profile_hook = lambda h: None
    sys.modules.setdefault('antenv.axon_hooks', mod)


def _build():
    import concourse.bass as bass
    import concourse.mybir as mybir
    import concourse.tile as tile
    from concourse import bacc
    from concourse.masks import make_identity

    f32 = mybir.dt.float32
    f32r = mybir.dt.float32r
    AF = mybir.ActivationFunctionType
    ALU = mybir.AluOpType

    nc = bacc.Bacc("TRN2", target_bir_lowering=False, debug=False, num_devices=N_CORES)

    q_in = nc.dram_tensor("q_in", [T, D], f32, kind="ExternalInput")
    c_in = nc.dram_tensor("c_in", [S_KEEP, D], f32, kind="ExternalInput")
    wq_in = nc.dram_tensor("wq_in", [DOUT, D], f32, kind="ExternalInput")
    wk_in = nc.dram_tensor("wk_in", [DOUT, D], f32, kind="ExternalInput")
    wv_in = nc.dram_tensor("wv_in", [DOUT, D], f32, kind="ExternalInput")
    wo_in = nc.dram_tensor("wo_in", [D, DOUT], f32, kind="ExternalInput")
    qres_in = nc.dram_tensor("qres_in", [TBW, D], f32, kind="ExternalInput")
    bias_in = nc.dram_tensor("bias_in", [NH, 4, 128, TBW], f32, kind="ExternalInput")
    rwb_in = nc.dram_tensor("rwb_in", [128, D], f32, kind="ExternalInput")
    y_out = nc.dram_tensor("y_out", [TBW, D], f32, kind="ExternalOutput")

    with tile.TileContext(nc) as tc:
        with tc.tile_pool(name="const", bufs=1) as cpool, \
             tc.tile_pool(name="srcp", bufs=4) as srcp, \
             tc.tile_pool(name="pers0", bufs=1) as pers0, \
             tc.tile_pool(name="dramp", bufs=1, space="DRAM") as dramp:

            ident = cpool.tile([128, 128], f32)
            make_identity(nc, ident)
            onesF = cpool.tile([128, 1], f32)
            nc.gpsimd.memset(onesF[:], 1.0)
            epsT = cpool.tile([128, 1], f32)
            nc.gpsimd.memset(epsT[:], EPS)

            woT = pers0.tile([128, 2, D], f32r)   # local slice: WoT[din_local 256, dout 1024]
            outT_sb = pers0.tile([128, 2, T], f32r)
            rwb = pers0.tile([128, D], f32)
            qres = pers0.tile([128, 2, D], f32)
            nc.sync.dma_start(rwb[:], rwb_in[:, :])
            nc.sync.dma_start(qres[:], qres_in.ap().rearrange("(m p) d -> p m d", p=128))

            rs_ins = [dramp.tile([TBW, D], f32, name=f"rs_in{t}") for t in range(NTB)]
            rs_outs = [dramp.tile([64, D], f32, name=f"rs_out{t}") for t in range(NTB)]

            _tctr = [0]

            def transpose_in(psT, dst, dram, nrows, ncols):
                # dst[p, k, r] = dram[r, 128k + p]; copies batched 4 transposes wide
                kgrp = 4 if ncols % 512 == 0 else 2
                for rt in range(nrows // 128):
                    st = srcp.tile([128, D], f32, tag="srcld", name="src_t")
                    nc.sync.dma_start(st[:, :ncols], dram[rt * 128:(rt + 1) * 128, :])
                    for k0 in range(0, ncols // 128, kgrp):
                        p = psT.tile([128, 512], f32, tag="tp", name="tp_ps")
                        for j in range(kgrp):
                            nc.tensor.transpose(p[:, j * 128:(j + 1) * 128],
                                                st[:, (k0 + j) * 128:(k0 + j + 1) * 128],
                                                ident[:])
                        eng = nc.vector if _tctr[0] % 3 < 2 else nc.scalar
                        _tctr[0] += 1
                        if eng is nc.vector:
                            eng.tensor_copy(
                                dst[:, k0:k0 + kgrp, rt * 128:(rt + 1) * 128],
                                p[:, :kgrp * 128].rearrange("p (j r) -> p j r", r=128))
                        else:
                            eng.copy(
                                dst[:, k0:k0 + kgrp, rt * 128:(rt + 1) * 128],
                                p[:, :kgrp * 128].rearrange("p (j r) -> p j r", r=128))

            with tc.tile_pool(name="persA", bufs=1) as persA:
                wqT = persA.tile([128, KSUB, DOUT], f32r)
                wkT = persA.tile([128, KSUB, DOUT], f32r)
                wvT = persA.tile([128, KSUB, DOUT], f32r)
                qT = persA.tile([128, 2, T], f32r)
                kT = persA.tile([128, 2, S_KEEP], f32r)
                vA = persA.tile([128, NST, NH, 128], f32r)   # [s | st | h | 64 v + 64 ones]
                bias_sb = persA.tile([128, NH, 4, TBW], f32)

                nc.sync.dma_start(bias_sb[:], bias_in.ap().rearrange("h d p j -> p h d j"))
                nc.vector.tensor_copy(vA[:, :, :, 64:128],
                                      onesF[:].to_broadcast((128, NST, NH, 64)))

                with tc.tile_pool(name="psT", bufs=2, space="PSUM") as psT:
                    transpose_in(psT, wqT, wq_in, DOUT, D)
                    transpose_in(psT, wkT, wk_in, DOUT, D)
                    transpose_in(psT, wvT, wv_in, DOUT, D)

                    with tc.tile_pool(name="bigT", bufs=1) as bigTp, \
                         tc.tile_pool(name="psP", bufs=2, space="PSUM") as psP:
                        bigQ = bigTp.tile([128, KSUB, S_KEEP], f32r, tag="big", name="bigQ")
                        transpose_in(psT, bigQ, q_in, T, D)  # uses [:, :, :1024]
                        # q projection (scale folded in)
                        for sub in range(2):
                            for nb in range(2):
                                pp = psP.tile([128, 512], f32, tag="proj", name="pp_q")
                                for k in range(KSUB):
                                    nc.tensor.matmul(pp[:], wqT[:, k, sub * 128:(sub + 1) * 128],
                                                     bigQ[:, k, nb * 512:(nb + 1) * 512],
                                                     start=(k == 0), stop=(k == KSUB - 1))
                                nc.scalar.mul(qT[:, sub, nb * 512:(nb + 1) * 512], pp[:], SCALE)

                        bigC = bigTp.tile([128, KSUB, S_KEEP], f32r, tag="big", name="bigC")
                        transpose_in(psT, bigC, c_in, S_KEEP, D)
                        # k projection
                        for sub in range(2):
                            for nb, (n0, nw) in enumerate(((0, 512), (512, 512), (1024, 128))):
                                pp = psP.tile([128, 512], f32, tag="proj", name="pp_k")
                                for k in range(KSUB):
                                    nc.tensor.matmul(pp[:, :nw], wkT[:, k, sub * 128:(sub + 1) * 128],
                                                     bigC[:, k, n0:n0 + nw],
                                                     start=(k == 0), stop=(k == KSUB - 1))
                                nc.any.tensor_copy(kT[:, sub, n0:n0 + nw], pp[:, :nw])
                        # v projection (natural layout, into vA's v-halves)
                        for st in range(NST):
                            pp = psP.tile([128, 512], f32, tag="proj", name="pp_v")
                            for k in range(KSUB):
                                nc.tensor.matmul(pp[:, :DOUT], bigC[:, k, st * 128:(st + 1) * 128],
                                                 wvT[:, k, :], start=(k == 0), stop=(k == KSUB - 1))
                            nc.any.tensor_copy(vA[:, st, :, 0:64],
                                               pp[:, :DOUT].rearrange("p (h e) -> p h e", e=64))

                    # Wo slice transposes (before attention; scheduler overlaps)
                    transpose_in(psT, woT, wo_in, D, DOUT)

                # ---- attention (tb-outer) + per-tb out-proj + chunked ReduceScatter ----
                with tc.tile_pool(name="expp", bufs=3) as expp, \
                     tc.tile_pool(name="tmpB", bufs=2) as tmpB, \
                     tc.tile_pool(name="stg", bufs=3) as stg, \
                     tc.tile_pool(name="psB", bufs=2, space="PSUM") as psB:
                    for tb in range(NTB):
                        sts = [st for st in range(2 * tb - 1, 2 * tb + 3) if st >= 0]
                        nst = len(sts)
                        d0 = sts[0] - (2 * tb - 1)
                        for pi in range(2):
                            den2 = tmpB.tile([128, TBW], f32, tag="den2", name="den2")
                            pos = []
                            for h in (2 * pi, 2 * pi + 1):
                                r0 = 64 * (h % 2)
                                sub = h // 2
                                ps_sc = psB.tile([128, 4, TBW], f32, tag="sc", name="ps_sc")
                                for i, st in enumerate(sts):
                                    nc.tensor.matmul(ps_sc[:, i, :],
                                                     kT[r0:r0 + 64, sub, st * 128:(st + 1) * 128],
                                                     qT[r0:r0 + 64, sub, tb * TBW:(tb + 1) * TBW],
                                                     start=True, stop=True)
                                nc.vector.tensor_tensor(ps_sc[:, :nst], ps_sc[:, :nst],
                                                        bias_sb[:, h, d0:d0 + nst, :], ALU.add)
                                exps = expp.tile([128, 4, TBW], f32r, tag="exps", name="exps")
                                nc.scalar.activation(exps[:, :nst], ps_sc[:, :nst], AF.Exp)
                                po = psB.tile([128, TBW], f32, tag=f"po{h % 2}", name="po")
                                for i, st in enumerate(sts):
                                    nc.tensor.matmul(po[:], vA[:, st, h, :], exps[:, i, :],
                                                     start=(i == 0), stop=(i == nst - 1))
                                nc.scalar.copy(den2[r0:r0 + 64, :], po[64:128, :])
                                pos.append(po)
                            rpk = tmpB.tile([128, TBW], f32, tag="rpk", name="rpk")
                            nc.vector.reciprocal(rpk[:], den2[:])
                            for j, h in enumerate((2 * pi, 2 * pi + 1)):
                                r0 = 64 * (h % 2)
                                sub = h // 2
                                nc.vector.tensor_tensor(
                                    outT_sb[r0:r0 + 64, sub, tb * TBW:(tb + 1) * TBW],
                                    pos[j][0:64, :], rpk[r0:r0 + 64, :], ALU.mult)
                        # local out-proj for this t-block, then ReduceScatter chunk
                        for mt in range(2):
                            tt = 2 * tb + mt
                            for nb in range(2):
                                pp = psB.tile([128, 512], f32, tag="sc", name="pp_o")
                                for k in range(2):
                                    nc.tensor.matmul(pp[:], outT_sb[:, k, tt * 128:(tt + 1) * 128],
                                                     woT[:, k, nb * 512:(nb + 1) * 512],
                                                     start=(k == 0), stop=(k == 1))
                                so = stg.tile([128, 512], f32, tag="postg", name="so")
                                nc.vector.tensor_copy(so[:], pp[:])
                                nc.sync.dma_start(
                                    rs_ins[tb][mt * 128:(mt + 1) * 128, nb * 512:(nb + 1) * 512],
                                    so[:])
                        nc.gpsimd.collective_compute(
                            "ReduceScatter", mybir.AluOpType.add,
                            ins=[rs_ins[tb].opt()], outs=[rs_outs[tb].opt()],
                            replica_groups=[[0, 1, 2, 3], [4, 5, 6, 7]])

            # ---- gather RS chunks, residual + rmsnorm (per mt as chunks arrive) ----
            with tc.tile_pool(name="persC", bufs=1) as persC:
                rq = persC.tile([128, 2, D], f32)
                for mt in range(2):
                    for tb in (2 * mt, 2 * mt + 1):
                        nc.sync.dma_start(rq[64 * (tb % 2):64 * (tb % 2) + 64, tb // 2, :],
                                          rs_outs[tb][:, :])
                    r = persC.tile([128, D], f32, tag="resid", name="resid")
                    nc.vector.tensor_tensor(r[:], rq[:, mt, :], qres[:, mt, :], ALU.add)
                    sq = persC.tile([128, D], f32, tag="sq", name="sq")
                    ms = persC.tile([128, 1], f32, tag="ms", name="ms")
                    nc.scalar.activation(sq[:], r[:], AF.Square, accum_out=ms[:])
                    rstd = persC.tile([128, 1], f32, tag="rstd", name="rstd")
                    nc.scalar.activation(rstd[:], ms[:], AF.Sqrt, scale=1.0 / D, bias=epsT[:])
                    rinv = persC.tile([128, 1], f32, tag="rinv", name="rinv")
                    nc.vector.reciprocal(rinv[:], rstd[:])
                    y1 = persC.tile([128, D], f32, tag="y1", name="y1")
                    nc.vector.tensor_scalar_mul(y1[:], r[:], rinv[:])
                    y2 = persC.tile([128, D], f32, tag="y2", name="y2")
                    nc.vector.tensor_tensor(y2[:], y1[:], rwb[:], ALU.mult)
                    nc.sync.dma_start(y_out[mt * 128:(mt + 1) * 128, :], y2[:])

    nc.compile()
    return nc


def _bias_tiles(g: int) -> np.ndarray:
    """Per-core host-precomputed alibi band tiles [NH, 4 deltas, 128, TBW]."""
    j = np.arange(TBW, dtype=np.float64)[None, :]
    p = np.arange(128, dtype=np.float64)[:, None]
    out = np.empty((NH, 4, 128, TBW), dtype=np.float32)
    for hl in range(NH):
        slope = float(SLOPES[4 * g + hl])
        for d in range(4):
            delta = 128 * (d - 1)
            out[hl, d] = (-slope * np.abs(j - p - delta)).astype(np.float32)
    return out


def kernel(query, context, Wq, Wk, Wv, Wo, rms_weight):
    global LAST_EXEC_NS
    query = np.ascontiguousarray(np.asarray(query, dtype=np.float32))
    context = np.ascontiguousarray(np.asarray(context, dtype=np.float32))
    Wq = np.ascontiguousarray(np.asarray(Wq, dtype=np.float32))
    Wk = np.ascontiguousarray(np.asarray(Wk, dtype=np.float32))
    Wv = np.ascontiguousarray(np.asarray(Wv, dtype=np.float32))
    Wo = np.ascontiguousarray(np.asarray(Wo, dtype=np.float32))
    rms_weight = np.asarray(rms_weight, dtype=np.float32)

    if TRACE:
        _install_ntff_shim()
    if "nc" not in _CACHE:
        _CACHE["nc"] = _build()
    nc = _CACHE["nc"]

    from concourse.bass_utils import run_bass_kernel_spmd

    rwb = np.ascontiguousarray(np.broadcast_to(rms_weight, (128, D))).astype(np.float32)
    in_maps = []
    for c in range(N_CORES):
        b, g = divmod(c, 4)
        rows = _row_map(g)
        in_maps.append({
            "q_in": query[b],
            "c_in": context[b, :S_KEEP],
            "wq_in": Wq[DOUT * g:DOUT * (g + 1)],
            "wk_in": Wk[DOUT * g:DOUT * (g + 1)],
            "wv_in": Wv[DOUT * g:DOUT * (g + 1)],
            "wo_in": np.ascontiguousarray(Wo[:, DOUT * g:DOUT * (g + 1)]),
            "qres_in": np.ascontiguousarray(query[b, rows]),
            "bias_in": _bias_tiles(g),
            "rwb_in": rwb,
        })

    res = run_bass_kernel_spmd(nc, in_maps, core_ids=list(range(N_CORES)), trace=TRACE)
    LAST_EXEC_NS = res.exec_time_ns
    _CACHE["last_result"] = res

    out = np.empty((B, T, D), dtype=np.float32)
    for c in range(N_CORES):
        b, g = divmod(c, 4)
        out[b, _row_map(g), :] = res.results[c]["y_out"]
    return out


# revision 17
# speedup vs baseline: 11110.2838x; 1.1866x over previous
"""Trainium2 Bass kernel for nn_AttentionBridge (ALiBi cross-attention + out-proj + rmsnorm residual).

Sharding: 8 cores = 2 (batch) x 4 (head groups of 4 heads).
Core c: batch b = c//4, head group g = c%4 (global heads 4g..4g+3).

Math per core (all in the "transposed domain" so no activation transposes are
needed after the initial query^T/context^T):
  queryT/ctxT via PE transposes -> f32r projections
  qT[dout,t] = WqT.T @ queryT   (scale 1/8 folded in)
  kT[dout,s], v[s,dout] similarly
  scoresT[s,t] = kT_h.T @ qT_h  (K=64) + alibi bias (host-precomputed band tiles)
  expsT = exp(scoresT)          (banded: only |t-s| <= ~128..384 computed;
                                 ALiBi slopes 0.5..0.96 make the rest < e^-60)
  PV with lhsT = [v_h | ones]: psum rows 0:64 = out, rows 64:128 = softmax denom
  normalize -> outT_g slice for that t-block
  per t-block: local out-proj (Wo column slice) + chunked ReduceScatter(add)
  within the batch group (overlaps later t-blocks' attention), then
  residual + rmsnorm on this core's rows.
Host assembles [2,1024,1024] from the per-core row sets.
"""
import os
import sys

for _p in ("/opt/trn_rl_repo", "/root/.axon_site/_ro/trn_rl_repo"):
    if os.path.isdir(_p) and _p not in sys.path:
        sys.path.insert(0, _p)

import numpy as np

# ---- problem constants (hardcoded per contract) ----
B, T, S_FULL, D = 2, 1024, 2048, 1024
H, HD = 16, 64
NH = 4              # heads per core
DOUT = NH * HD      # 256 projected dims per core
SCALE = HD ** -0.5  # 0.125
EPS = 1e-6
N_CORES = 8
KSUB = D // 128     # 8 contraction subtiles
NST = 9             # context s-tiles kept (s < 1152; beyond that alibi bias < -400)
S_KEEP = NST * 128  # 1152
NTB = 4             # t-blocks of 256
TBW = 256
SLOPES = (1.0 / np.power(2.0, np.arange(1, H + 1, dtype=np.float64) / H)).astype(np.float32)

TRACE = False
LAST_EXEC_NS = None
_CACHE = {}


def _row_map(g: int) -> np.ndarray:
    """Global t row held at local row r (0..255) of the per-core result.

    Local rows are [mt 2][p 128]; chunk tb = 2*mt + p//64; within = p%64;
    global t = 256*tb + 64*g + within (g = rank in the 4-core RS group).
    """
    r = np.arange(2 * 128)
    mt, p = r // 128, r % 128
    tb = 2 * mt + p // 64
    return 256 * tb + 64 * g + (p % 64)


def _install_ntff_shim():
    """Optional: register the NTFF profile hook so trace=True works under axon."""
    import types
    try:
        from trn_agent_boot.trn_boot import _ntff_profile_via_ctypes
        hook = _ntff_profile_via_ctypes('/opt/axon/libaxon_pjrt.so')
    except Exception:
        hook = None
    mod = types.ModuleType('antenv.axon_hooks')
    mod.get_axon_ntff_profile_hook = lambda: hook
    mod.set_axon_ntff_profile_hook = lambda h: None
    sys.modules.setdefault('antenv.axon_hooks', mod)


def _build():
    import concourse.bass as bass
    import concourse.mybir as mybir
    import concourse.tile as tile
    from concourse import bacc
    from concourse.masks import make_identity

    f32 = mybir.dt.float32
    f32r = mybir.dt.float32r
    AF = mybir.ActivationFunctionType
    ALU = mybir.AluOpType

    nc = bacc.Bacc("TRN2", target_bir_lowering=False, debug=False, num_devices=N_CORES)

    q_in = nc.dram_tensor("q_in", [T, D], f32, kind="ExternalInput")
    c_in = nc.dram_tensor("c_in", [S_KEEP, D], f32, kind="ExternalInput")
    wq_in = nc.dram_tensor("wq_in", [DOUT, D], f32, kind="ExternalInput")
    wk_in = nc.dram_tensor("wk_in", [DOUT, D], f32, kind="ExternalInput")
    wv_in = nc.dram_tensor("wv_in", [DOUT, D], f32, kind="ExternalInput")
    wo_in = nc.dram_tensor("wo_in", [D, DOUT], f32, kind="ExternalInput")
    qres_in = nc.dram_tensor("qres_in", [TBW, D], f32, kind="ExternalInput")
    bias_in = nc.dram_tensor("bias_in", [NH, 4, 128, TBW], f32, kind="ExternalInput")
    rwb_in = nc.dram_tensor("rwb_in", [128, D], f32, kind="ExternalInput")
    y_out = nc.dram_tensor("y_out", [TBW, D], f32, kind="ExternalOutput")

    with tile.TileContext(nc) as tc:
        with tc.tile_pool(name="const", bufs=1) as cpool, \
             tc.tile_pool(name="srcp", bufs=4) as srcp, \
             tc.tile_pool(name="pers0", bufs=1) as pers0, \
             tc.tile_pool(name="dramp", bufs=1, space="DRAM") as dramp:

            ident = cpool.tile([128, 128], f32)
            make_identity(nc, ident)
            onesF = cpool.tile([128, 1], f32)
            nc.gpsimd.memset(onesF[:], 1.0)
            epsT = cpool.tile([128, 1], f32)
            nc.gpsimd.memset(epsT[:], EPS)

            woT = pers0.tile([128, 2, D], f32r)   # local slice: WoT[din_local 256, dout 1024]
            outT_sb = pers0.tile([128, 2, T], f32r)
            rwb = pers0.tile([128, D], f32)
            qres = pers0.tile([128, 2, D], f32)
            nc.sync.dma_start(rwb[:], rwb_in[:, :])
            nc.sync.dma_start(qres[:], qres_in.ap().rearrange("(m p) d -> p m d", p=128))

            bf16 = __import__("concourse.mybir", fromlist=["dt"]).dt.bfloat16
            rs_ins = [dramp.tile([TBW, D], bf16, name=f"rs_in{t}") for t in range(NTB)]
            rs_outs = [dramp.tile([64, D], bf16, name=f"rs_out{t}") for t in range(NTB)]

            _tctr = [0]

            def transpose_in(psT, dst, dram, nrows, ncols):
                # dst[p, k, r] = dram[r, 128k + p]; copies batched 4 transposes wide
                kgrp = 4 if ncols % 512 == 0 else 2
                for rt in range(nrows // 128):
                    st = srcp.tile([128, D], f32, tag="srcld", name="src_t")
                    nc.sync.dma_start(st[:, :ncols], dram[rt * 128:(rt + 1) * 128, :])
                    for k0 in range(0, ncols // 128, kgrp):
                        p = psT.tile([128, 512], f32, tag="tp", name="tp_ps")
                        for j in range(kgrp):
                            nc.tensor.transpose(p[:, j * 128:(j + 1) * 128],
                                                st[:, (k0 + j) * 128:(k0 + j + 1) * 128],
                                                ident[:])
                        eng = nc.vector if _tctr[0] % 3 < 2 else nc.scalar
                        _tctr[0] += 1
                        if eng is nc.vector:
                            eng.tensor_copy(
                                dst[:, k0:k0 + kgrp, rt * 128:(rt + 1) * 128],
                                p[:, :kgrp * 128].rearrange("p (j r) -> p j r", r=128))
                        else:
                            eng.copy(
                                dst[:, k0:k0 + kgrp, rt * 128:(rt + 1) * 128],
                                p[:, :kgrp * 128].rearrange("p (j r) -> p j r", r=128))

            with tc.tile_pool(name="persA", bufs=1) as persA:
                wqT = persA.tile([128, KSUB, DOUT], f32r)
                wkT = persA.tile([128, KSUB, DOUT], f32r)
                wvT = persA.tile([128, KSUB, DOUT], f32r)
                qT = persA.tile([128, 2, T], f32r)
                kT = persA.tile([128, 2, S_KEEP], f32r)
                vA = persA.tile([128, NST, NH, 128], f32r)   # [s | st | h | 64 v + 64 ones]
                bias_sb = persA.tile([128, NH, 4, TBW], f32)

                nc.sync.dma_start(bias_sb[:], bias_in.ap().rearrange("h d p j -> p h d j"))
                nc.vector.tensor_copy(vA[:, :, :, 64:128],
                                      onesF[:].to_broadcast((128, NST, NH, 64)))

                with tc.tile_pool(name="psT", bufs=2, space="PSUM") as psT:
                    transpose_in(psT, wqT, wq_in, DOUT, D)
                    transpose_in(psT, wkT, wk_in, DOUT, D)
                    transpose_in(psT, wvT, wv_in, DOUT, D)

                    with tc.tile_pool(name="bigT", bufs=1) as bigTp, \
                         tc.tile_pool(name="psP", bufs=2, space="PSUM") as psP:
                        bigQ = bigTp.tile([128, KSUB, S_KEEP], f32r, tag="big", name="bigQ")
                        transpose_in(psT, bigQ, q_in, T, D)  # uses [:, :, :1024]
                        # q projection (scale folded in)
                        for sub in range(2):
                            for nb in range(2):
                                pp = psP.tile([128, 512], f32, tag="proj", name="pp_q")
                                for k in range(KSUB):
                                    nc.tensor.matmul(pp[:], wqT[:, k, sub * 128:(sub + 1) * 128],
                                                     bigQ[:, k, nb * 512:(nb + 1) * 512],
                                                     start=(k == 0), stop=(k == KSUB - 1))
                                nc.scalar.mul(qT[:, sub, nb * 512:(nb + 1) * 512], pp[:], SCALE)

                        bigC = bigTp.tile([128, KSUB, S_KEEP], f32r, tag="big", name="bigC")
                        transpose_in(psT, bigC, c_in, S_KEEP, D)
                        # k projection
                        for sub in range(2):
                            for nb, (n0, nw) in enumerate(((0, 512), (512, 512), (1024, 128))):
                                pp = psP.tile([128, 512], f32, tag="proj", name="pp_k")
                                for k in range(KSUB):
                                    nc.tensor.matmul(pp[:, :nw], wkT[:, k, sub * 128:(sub + 1) * 128],
                                                     bigC[:, k, n0:n0 + nw],
                                                     start=(k == 0), stop=(k == KSUB - 1))
                                nc.any.tensor_copy(kT[:, sub, n0:n0 + nw], pp[:, :nw])
                        # v projection (natural layout, into vA's v-halves)
                        for st in range(NST):
                            pp = psP.tile([128, 512], f32, tag="proj", name="pp_v")
                            for k in range(KSUB):
                                nc.tensor.matmul(pp[:, :DOUT], bigC[:, k, st * 128:(st + 1) * 128],
                                                 wvT[:, k, :], start=(k == 0), stop=(k == KSUB - 1))
                            nc.any.tensor_copy(vA[:, st, :, 0:64],
                                               pp[:, :DOUT].rearrange("p (h e) -> p h e", e=64))

                    # Wo slice transposes (before attention; scheduler overlaps)
                    transpose_in(psT, woT, wo_in, D, DOUT)

                # ---- attention (tb-outer) + per-tb out-proj + chunked ReduceScatter ----
                with tc.tile_pool(name="expp", bufs=3) as expp, \
                     tc.tile_pool(name="tmpB", bufs=2) as tmpB, \
                     tc.tile_pool(name="stg", bufs=3) as stg, \
                     tc.tile_pool(name="psB", bufs=2, space="PSUM") as psB:
                    for tb in range(NTB):
                        sts = [st for st in range(2 * tb - 1, 2 * tb + 3) if st >= 0]
                        nst = len(sts)
                        d0 = sts[0] - (2 * tb - 1)
                        for pi in range(2):
                            den2 = tmpB.tile([128, TBW], f32, tag="den2", name="den2")
                            pos = []
                            for h in (2 * pi, 2 * pi + 1):
                                r0 = 64 * (h % 2)
                                sub = h // 2
                                ps_sc = psB.tile([128, 4, TBW], f32, tag="sc", name="ps_sc")
                                for i, st in enumerate(sts):
                                    nc.tensor.matmul(ps_sc[:, i, :],
                                                     kT[r0:r0 + 64, sub, st * 128:(st + 1) * 128],
                                                     qT[r0:r0 + 64, sub, tb * TBW:(tb + 1) * TBW],
                                                     start=True, stop=True)
                                nc.vector.tensor_tensor(ps_sc[:, :nst], ps_sc[:, :nst],
                                                        bias_sb[:, h, d0:d0 + nst, :], ALU.add)
                                exps = expp.tile([128, 4, TBW], f32r, tag="exps", name="exps")
                                nc.scalar.activation(exps[:, :nst], ps_sc[:, :nst], AF.Exp)
                                po = psB.tile([128, TBW], f32, tag=f"po{h % 2}", name="po")
                                for i, st in enumerate(sts):
                                    nc.tensor.matmul(po[:], vA[:, st, h, :], exps[:, i, :],
                                                     start=(i == 0), stop=(i == nst - 1))
                                nc.scalar.copy(den2[r0:r0 + 64, :], po[64:128, :])
                                pos.append(po)
                            rpk = tmpB.tile([128, TBW], f32, tag="rpk", name="rpk")
                            nc.vector.reciprocal(rpk[:], den2[:])
                            for j, h in enumerate((2 * pi, 2 * pi + 1)):
                                r0 = 64 * (h % 2)
                                sub = h // 2
                                nc.vector.tensor_tensor(
                                    outT_sb[r0:r0 + 64, sub, tb * TBW:(tb + 1) * TBW],
                                    pos[j][0:64, :], rpk[r0:r0 + 64, :], ALU.mult)
                        # local out-proj for this t-block, then ReduceScatter chunk
                        for mt in range(2):
                            tt = 2 * tb + mt
                            for nb in range(2):
                                pp = psB.tile([128, 512], f32, tag="sc", name="pp_o")
                                for k in range(2):
                                    nc.tensor.matmul(pp[:], outT_sb[:, k, tt * 128:(tt + 1) * 128],
                                                     woT[:, k, nb * 512:(nb + 1) * 512],
                                                     start=(k == 0), stop=(k == 1))
                                so = stg.tile([128, 512], bf16, tag="postg", name="so")
                                nc.scalar.copy(so[:], pp[:])
                                nc.sync.dma_start(
                                    rs_ins[tb][mt * 128:(mt + 1) * 128, nb * 512:(nb + 1) * 512],
                                    so[:])
                        nc.gpsimd.collective_compute(
                            "ReduceScatter", mybir.AluOpType.add,
                            ins=[rs_ins[tb].opt()], outs=[rs_outs[tb].opt()],
                            replica_groups=[[0, 1, 2, 3], [4, 5, 6, 7]])

            # ---- gather RS chunks, residual + rmsnorm (per mt as chunks arrive) ----
            with tc.tile_pool(name="persC", bufs=1) as persC:
                rq = persC.tile([128, 2, D], bf16)
                for mt in range(2):
                    for tb in (2 * mt, 2 * mt + 1):
                        nc.sync.dma_start(rq[64 * (tb % 2):64 * (tb % 2) + 64, tb // 2, :],
                                          rs_outs[tb][:, :])
                    r = persC.tile([128, D], f32, tag="resid", name="resid")
                    nc.vector.tensor_tensor(r[:], rq[:, mt, :], qres[:, mt, :], ALU.add)
                    sq = persC.tile([128, D], f32, tag="sq", name="sq")
                    ms = persC.tile([128, 1], f32, tag="ms", name="ms")
                    nc.scalar.activation(sq[:], r[:], AF.Square, accum_out=ms[:])
                    rstd = persC.tile([128, 1], f32, tag="rstd", name="rstd")
                    nc.scalar.activation(rstd[:], ms[:], AF.Sqrt, scale=1.0 / D, bias=epsT[:])
                    rinv = persC.tile([128, 1], f32, tag="rinv", name="rinv")
                    nc.vector.reciprocal(rinv[:], rstd[:])
                    y1 = persC.tile([128, D], f32, tag="y1", name="y1")
                    nc.vector.tensor_scalar_mul(y1[:], r[:], rinv[:])
                    y2 = persC.tile([128, D], f32, tag="y2", name="y2")
                    nc.vector.tensor_tensor(y2[:], y1[:], rwb[:], ALU.mult)
                    nc.sync.dma_start(y_out[mt * 128:(mt + 1) * 128, :], y2[:])

    nc.compile()
    return nc


def _bias_tiles(g: int) -> np.ndarray:
    """Per-core host-precomputed alibi band tiles [NH, 4 deltas, 128, TBW]."""
    j = np.arange(TBW, dtype=np.float64)[None, :]
    p = np.arange(128, dtype=np.float64)[:, None]
    out = np.empty((NH, 4, 128, TBW), dtype=np.float32)
    for hl in range(NH):
        slope = float(SLOPES[4 * g + hl])
        for d in range(4):
            delta = 128 * (d - 1)
            out[hl, d] = (-slope * np.abs(j - p - delta)).astype(np.float32)
    return out


def kernel(query, context, Wq, Wk, Wv, Wo, rms_weight):
    global LAST_EXEC_NS
    query = np.ascontiguousarray(np.asarray(query, dtype=np.float32))
    context = np.ascontiguousarray(np.asarray(context, dtype=np.float32))
    Wq = np.ascontiguousarray(np.asarray(Wq, dtype=np.float32))
    Wk = np.ascontiguousarray(np.asarray(Wk, dtype=np.float32))
    Wv = np.ascontiguousarray(np.asarray(Wv, dtype=np.float32))
    Wo = np.ascontiguousarray(np.asarray(Wo, dtype=np.float32))
    rms_weight = np.asarray(rms_weight, dtype=np.float32)

    if TRACE:
        _install_ntff_shim()
    if "nc" not in _CACHE:
        _CACHE["nc"] = _build()
    nc = _CACHE["nc"]

    from concourse.bass_utils import run_bass_kernel_spmd

    rwb = np.ascontiguousarray(np.broadcast_to(rms_weight, (128, D))).astype(np.float32)
    in_maps = []
    for c in range(N_CORES):
        b, g = divmod(c, 4)
        rows = _row_map(g)
        in_maps.append({
            "q_in": query[b],
            "c_in": context[b, :S_KEEP],
            "wq_in": Wq[DOUT * g:DOUT * (g + 1)],
            "wk_in": Wk[DOUT * g:DOUT * (g + 1)],
            "wv_in": Wv[DOUT * g:DOUT * (g + 1)],
            "wo_in": np.ascontiguousarray(Wo[:, DOUT * g:DOUT * (g + 1)]),
            "qres_in": np.ascontiguousarray(query[b, rows]),
            "bias_in": _bias_tiles(g),
            "rwb_in": rwb,
        })

    res = run_bass_kernel_spmd(nc, in_maps, core_ids=list(range(N_CORES)), trace=TRACE)
    LAST_EXEC_NS = res.exec_time_ns
    _CACHE["last_result"] = res

    out = np.empty((B, T, D), dtype=np.float32)
    for c in range(N_CORES):
        b, g = divmod(c, 4)
        out[b, _row_map(g), :] = res.results[c]["y_out"]
    return out
